# revision 1
# baseline (speedup 1.0000x reference)
"""Trainium2 Bass kernel for the non-local attention block (dense_transformer).

Reference computation per batch item b (x: [B=32, C=64, H=32, W=32], N=1024):
    xf    = x[b] reshaped [C, N]
    phi   = w_phi   @ xf                     [C, N]
    theta = (w_theta @ xf)^T                 [N, C]
    g     = (w_g @ xf)^T @ w_mv^T            [N, C]
    att   = theta @ phi                      [N, N]
    att   = att @ w_mk^T                     [N, N]
    att   = softmax(att, axis over rows n)
    out   = att @ g                          [N, C]
    final = w_mask @ out^T + xf              [C, N]

Key algebraic restructure: (theta @ phi) @ w_mk^T == theta @ (phi @ w_mk^T),
which removes the N^3 matmul (1073M MACs -> 2x67M MACs per batch).  The
softmax denominator divide is folded into the small g factor (64 wide)
instead of the [N, N] attention matrix.

Per-core layout (data-parallel, 4 batch items per core, processed as 2
stacked pairs occupying the 128 SBUF partitions; batch "b" on partitions
0-63, batch "c" on 64-127, PE quadrant tile-position packing runs both
batches' matmuls concurrently):
    T    = w_theta @ xf          [64, 1024]  (diag-quadrant pair matmuls)
    PhiT = xf^T @ w_phi^T        [1024, 64]  (row-split pair matmuls)
    GT   = xf^T @ (w_mv@w_g)^T   [1024, 64]  (row-split)
    P2   = PhiT^T @ w_mk^T       [64, 1024]  (col-split, accum over 8 m-chunks)
    S    = P2^T @ T              [1024, 1024] = att2^T  (row-split per k-chunk)
    E    = exp(S)  (ScalarE, fused row-sum via accum_out -> D)
    GTs  = GT * (1/D)            (fold softmax divide into g)
    O    = GTs^T @ E             [64, 1024]  (col-split, accum over m-chunks)
    final= w_mask @ O + xf       (diag-quadrant + DVE add)

All matmul operands bf16 (PE full rate); PSUM accumulation fp32; softmax
sum in fp32 via activation accum_out.  Weights are pre-transposed/cast on
host and replicated to all 8 cores.

PSUM budget (8 banks): S/exp pipeline 2 slots x [128,1024] = 4 banks;
P2 quarter-chunks + O accumulators share a 2-slot pool = 2 banks;
stage-1/mask psums rotate through another 2-slot pool = 2 banks.

Post-passes: _eliminate_redundant_waits strips Tile's transitively-implied
same-engine sem waits (they serialize the PE pipeline and block quadrant
concurrency); _split_matmul_waits hoists remaining multi-wait instructions
onto single-wait NoOps (TRN2 walrus allows one sync-wait per instruction).

Measured on TRN2 via axon NTFF profile: 87-90 us.  The PE HAM clock
gate can throttle whole phases depending on run alignment and device
state; the explicit O-after-next-S dependency edges (see o_chunk) keep
cold O matmuls from head-of-line-blocking the exp-feeding S chain in the
PE FIFO, which cut the throttled-mode time from ~100 to ~90 us.
Rel err ~1.0e-2 vs the fp32 reference.
"""

import numpy as np
import ml_dtypes

import concourse.bass as bass
import concourse.mybir as mybir
import concourse.tile as tile
from concourse.bass_utils import run_bass_kernel_spmd

BF = mybir.dt.bfloat16
F32 = mybir.dt.float32
EXP = mybir.ActivationFunctionType.Exp

B, C, HH, WW = 32, 64, 32, 32
N = HH * WW          # 1024
NCORES = 8
BPC = B // NCORES    # 4 batch items per core
NPAIRS = BPC // 2    # 2 stacked pairs per core
NK = N // 128        # 8 chunks of 128 along the N dimension
NH = 512             # matmul free-dim half (one PSUM bank)


def _build_body(nc, tc, consts, acts, bigacts, psO_pool, psS, psSm,
                xall32, xall16, wsmallT, wmkhT, out_e):
    lo = slice(0, 64)
    hi = slice(64, 128)

    # ---- PE warmup: dummy matmuls on an uninitialized tile keep the PE
    # busy for the HAM SHORT window (~3.4us) while the input DMAs run, so
    # real work starts at 2.4 GHz instead of 1.2.
    warm_in = consts.tile([128, 256], BF, tag="warm_in")
    nc.gpsimd.memset(warm_in[:], 0.0)
    warm_ps = psS.tile([128, N], F32, tag="psS", name="warm_ps")
    for i in range(40):
        nc.tensor.matmul(warm_ps[:, 0:128], lhsT=warm_in[:, 0:128],
                         rhs=warm_in[:, 128:256])

    # ---- inputs: few large DMAs, split across SP and GpSimd SWDGE rings
    # (each dma_start costs ~1us of sequencer issue time).
    wsmall = consts.tile([128, 4 * C], BF, tag="wsmall")
    nc.sync.dma_start(wsmall[:], wsmallT[:])
    wth = wsmall[:, 0 * C:1 * C]
    wph = wsmall[:, 1 * C:2 * C]
    wgv = wsmall[:, 2 * C:3 * C]
    wma = wsmall[:, 3 * C:4 * C]

    # All input DMAs on the SP ring in priority order: HW queues serve
    # descriptors FIFO per queue, so earlier-pushed transfers complete
    # first.  xball gates stage-1, wmk h0 gates P2, xfall only the final
    # residual add.  (A second ring would interleave descriptors and delay
    # the critical x transfer.)
    xball = acts.tile([128, NPAIRS, N], BF, tag="xball")
    nc.sync.dma_start(xball[:], xall16.rearrange("(p q) n -> q p n", p=NPAIRS))
    # wmk^T in k-quarter-major DRAM layout [4, 1024(m), 256(k)]: one DMA
    # piece per quarter so P2 quarter j (and then S chunk 2j's exp) starts
    # as soon as piece j lands instead of waiting for the full 2MB.
    wmk_q = []
    for j in range(4):
        t = consts.tile([128, NK, 256], BF, tag=f"wmkq{j}")
        nc.sync.dma_start(
            t[:], wmkhT[j * N:(j + 1) * N, :].rearrange(
                "(mc q) k -> q mc k", mc=NK))
        wmk_q.append(t)
    xfall = acts.tile([128, NPAIRS, N], F32, tag="xfall")
    nc.sync.dma_start(xfall[:], xall32.rearrange("(p q) n -> q p n", p=NPAIRS))

    st = [dict() for _ in range(NPAIRS)]

    def stage1(p):
        """PhiT/T/GT (quadrant-packed) for pair p."""
        xb = xball[:, p, :]
        s = st[p]
        psPhiT_b = psSm.tile([128, NH], F32, tag="psSm", name="psPhiT_b")
        psPhiT_c = psSm.tile([128, NH], F32, tag="psSm", name="psPhiT_c")
        for m in range(NK):
            mm = slice(m * 128, (m + 1) * 128)
            cc = slice(m * C, (m + 1) * C)
            nc.tensor.matmul(psPhiT_b[:, cc], lhsT=xb[lo, mm], rhs=wph[lo, :])
            nc.tensor.matmul(psPhiT_c[:, cc], lhsT=xb[hi, mm], rhs=wph[hi, :])
        PhiT_b = acts.tile([128, NH], BF, tag="PhiT_b", name="PhiT_b")
        PhiT_c = acts.tile([128, NH], BF, tag="PhiT_c", name="PhiT_c")
        nc.vector.tensor_copy(out=PhiT_b[:], in_=psPhiT_b[:])
        nc.vector.tensor_copy(out=PhiT_c[:], in_=psPhiT_c[:])

        T_sb = acts.tile([128, N], BF, tag="T_sb", name="T_sb")
        for h in range(2):
            hh = slice(h * NH, (h + 1) * NH)
            psT = psSm.tile([128, NH], F32, tag="psSm", name="psT")
            nc.tensor.matmul(psT[lo, :], lhsT=wth[lo, :], rhs=xb[lo, hh])
            nc.tensor.matmul(psT[hi, :], lhsT=wth[hi, :], rhs=xb[hi, hh])
            nc.vector.tensor_copy(out=T_sb[:, hh], in_=psT[:])

        s.update(T_sb=T_sb, PhiT_b=PhiT_b, PhiT_c=PhiT_c)
        s["P2"] = acts.tile([128, N], BF, tag="P2", name="P2")

    def gtstage(p):
        """GT for pair p — off the first-exp critical path."""
        xb = xball[:, p, :]
        s = st[p]
        psGT_b = psSm.tile([128, NH], F32, tag="psSm", name="psGT_b")
        psGT_c = psSm.tile([128, NH], F32, tag="psSm", name="psGT_c")
        for m in range(NK):
            mm = slice(m * 128, (m + 1) * 128)
            cc = slice(m * C, (m + 1) * C)
            nc.tensor.matmul(psGT_b[:, cc], lhsT=xb[lo, mm], rhs=wgv[lo, :])
            nc.tensor.matmul(psGT_c[:, cc], lhsT=xb[hi, mm], rhs=wgv[hi, :])
        GT_b = acts.tile([128, NH], BF, tag="GT_b", name="GT_b")
        GT_c = acts.tile([128, NH], BF, tag="GT_c", name="GT_c")
        nc.vector.tensor_copy(out=GT_b[:], in_=psGT_b[:])
        nc.vector.tensor_copy(out=GT_c[:], in_=psGT_c[:])
        s.update(GT_b=GT_b, GT_c=GT_c)

    def p2_quarter(p, j):
        """P2 column-quarter j (256 k's) for pair p, col-split by batch."""
        s = st[p]
        jj = slice(j * 256, (j + 1) * 256)
        psP2 = psO_pool.tile([128, 256], F32, tag="psO", name="psP2")
        for m in range(NK):
            cc = slice(m * C, (m + 1) * C)
            nc.tensor.matmul(psP2[lo, :], lhsT=acts_slice(s, "PhiT_b", cc),
                             rhs=wmk_q[j][:, m, :],
                             start=(m == 0), stop=(m == NK - 1))
            nc.tensor.matmul(psP2[hi, :], lhsT=acts_slice(s, "PhiT_c", cc),
                             rhs=wmk_q[j][:, m, :],
                             start=(m == 0), stop=(m == NK - 1))
        nc.vector.tensor_copy(out=s["P2"][:, jj], in_=psP2[:])

    def acts_slice(s, key, cc):
        return s[key][:, cc]

    def alloc_e(p):
        s = st[p]
        s["E_b"] = bigacts.tile([128, NK, N], BF, tag="E_b", name="E_b")
        s["E_c"] = bigacts.tile([128, NK, N], BF, tag="E_c", name="E_c")
        s["D_b"] = acts.tile([128, NK], F32, tag="D_b", name="D_b")
        s["D_c"] = acts.tile([128, NK], F32, tag="D_c", name="D_c")
        s["R_b"] = acts.tile([128, NK], F32, tag="R_b", name="R_b")
        s["R_c"] = acts.tile([128, NK], F32, tag="R_c", name="R_c")
        s["GTs_b"] = acts.tile([128, NH], BF, tag="GTs_b", name="GTs_b")
        s["GTs_c"] = acts.tile([128, NH], BF, tag="GTs_c", name="GTs_c")

    def s_exp_chunk(p, k):
        """S matmuls + exp (fused row-sum) for k-chunk of pair p.

        Each batch's [128, 512] matmul is col-split into two M=64 pieces so
        all four PE quadrants run concurrently (row-only-split matmul pairs
        do NOT overlap — col groups get their own XBUS streams, row groups
        share one).  Output layout in PSUM is unchanged: partition q of the
        chunk still holds k-index k*128+q.
        """
        s = st[p]
        klo = slice(k * 128, k * 128 + 64)
        khi = slice(k * 128 + 64, (k + 1) * 128)
        psS_b = psS.tile([128, N], F32, tag="psS", name="psS_b")
        psS_c = psS.tile([128, N], F32, tag="psS", name="psS_c")
        last_s_mm = [None]
        for h in range(2):
            hh = slice(h * NH, (h + 1) * NH)
            nc.tensor.matmul(psS_b[lo, hh], lhsT=s["P2"][lo, klo],
                             rhs=s["T_sb"][lo, hh])
            nc.tensor.matmul(psS_b[hi, hh], lhsT=s["P2"][lo, khi],
                             rhs=s["T_sb"][lo, hh])
            nc.tensor.matmul(psS_c[lo, hh], lhsT=s["P2"][hi, klo],
                             rhs=s["T_sb"][hi, hh])
            last_s_mm[0] = nc.tensor.matmul(
                psS_c[hi, hh], lhsT=s["P2"][hi, khi],
                rhs=s["T_sb"][hi, hh])
        nc.scalar.activation(s["E_b"][:, k, :], psS_b[:], EXP,
                             accum_out=s["D_b"][:, k:k + 1])
        nc.scalar.activation(s["E_c"][:, k, :], psS_c[:], EXP,
                             accum_out=s["D_c"][:, k:k + 1])
        return last_s_mm[0]

    def gts_chunkwise_init(p):
        """Allocate pair p's O accumulator banks (R/GTs live in alloc_e)."""
        s = st[p]
        s["psO"] = [psO_pool.tile([128, NH], F32, tag="psO", name=f"psO{h}")
                    for h in range(2)]

    def gts_chunk(p, k):
        s = st[p]
        cc = slice(k * C, (k + 1) * C)
        nc.vector.reciprocal(s["R_b"][:, k:k + 1], s["D_b"][:, k:k + 1])
        nc.vector.reciprocal(s["R_c"][:, k:k + 1], s["D_c"][:, k:k + 1])
        nc.vector.tensor_scalar_mul(s["GTs_b"][:, cc], s["GT_b"][:, cc],
                                    s["R_b"][:, k:k + 1])
        nc.vector.tensor_scalar_mul(s["GTs_c"][:, cc], s["GT_c"][:, cc],
                                    s["R_c"][:, k:k + 1])

    def o_chunk(p, m, after=None):
        """O accumulation m-chunk for pair p (both halves, col-split).
        `after`: instruction the first O matmul must follow in the PE
        stream (the scheduler's warm-timing model otherwise places cold O
        matmuls ahead of the next S chunk, stalling the exp chain)."""
        from concourse.bass import _add_dep_helper
        s = st[p]
        cc = slice(m * C, (m + 1) * C)
        for h in range(2):
            hh = slice(h * NH, (h + 1) * NH)
            mm1 = nc.tensor.matmul(s["psO"][h][lo, :], lhsT=s["GTs_b"][:, cc],
                                   rhs=s["E_b"][:, m, hh],
                                   start=(m == 0), stop=(m == NK - 1))
            if after is not None:
                _add_dep_helper(mm1.ins, after.ins,
                                reason="O chunk after next S chunk")
                after = None
            nc.tensor.matmul(s["psO"][h][hi, :], lhsT=s["GTs_c"][:, cc],
                             rhs=s["E_c"][:, m, hh],
                             start=(m == 0), stop=(m == NK - 1))

    def finish(p):
        """O copyback, mask, residual add, out DMA for pair p."""
        s = st[p]
        O_sb = acts.tile([128, N], BF, tag="O_sb", name="O_sb")
        for h in range(2):
            hh = slice(h * NH, (h + 1) * NH)
            nc.vector.tensor_copy(out=O_sb[:, hh], in_=s["psO"][h][:])
        out_sb = acts.tile([128, N], F32, tag="out_sb", name="out_sb")
        for h in range(2):
            hh = slice(h * NH, (h + 1) * NH)
            psM = psSm.tile([128, NH], F32, tag="psSm", name="psM")
            nc.tensor.matmul(psM[lo, :], lhsT=wma[lo, :], rhs=O_sb[lo, hh])
            nc.tensor.matmul(psM[hi, :], lhsT=wma[hi, :], rhs=O_sb[hi, hh])
            nc.vector.tensor_tensor(out_sb[:, hh], psM[:],
                                    xfall[:, p, hh], mybir.AluOpType.add)
        nc.gpsimd.dma_start(out_e[p * 128:(p + 1) * 128, :], out_sb[:])

    # ---- software pipeline over the pairs ----
    # Pair 0's O rides pair 1's exp phase; pair 1's O runs in the tail.
    # Next pair's stage-1/P2 fills the current phase at low priority.
    # NOTE: DVE is strict FIFO — any DVE op whose producer resolves late
    # head-of-line-blocks later critical copies, so GT stays at normal
    # priority right after stage-1.
    def low():
        return tc.high_priority(offset=-100000)

    stage1(0)
    gtstage(0)
    alloc_e(0)
    gts_chunkwise_init(0)
    for j in range(4):
        p2_quarter(0, j)
    for p in range(NPAIRS):
        nxt = p + 1
        for k in range(NK):
            s_mm = s_exp_chunk(p, k)
            gts_chunk(p, k)
            if p > 0:
                with low():
                    if k >= 1:
                        o_chunk(p - 1, k - 1, after=s_mm)
                    if k == NK - 1:
                        o_chunk(p - 1, NK - 1, after=None)
            if nxt < NPAIRS:
                if k == 1:
                    with low():
                        stage1(nxt)
                        gtstage(nxt)
                if k == 3:
                    alloc_e(nxt)
                if 2 <= k < 6:
                    with low():
                        p2_quarter(nxt, k - 2)
            if p > 0 and k == NK - 1:
                with low():
                    finish(p - 1)
        if nxt < NPAIRS:
            gts_chunkwise_init(nxt)
    for m in range(NK):
        o_chunk(NPAIRS - 1, m)
    finish(NPAIRS - 1)


def _eliminate_redundant_waits(nc):
    """Transitive redundant-wait elimination over the final BIR stream.

    Tile's sem assignment is per-proc minimal but NOT transitively minimal:
    e.g. a matmul reusing a PSUM slot gets both (ACT >= k) [reader done] and
    (PE >= p) [previous writer done] waits, although observing ACT >= k
    already implies PE >= p (the reader waited on the writer).  The extra
    same-engine waits serialize the PE pipeline (no back-to-back streaming,
    no quadrant concurrency).

    Soundness relies on per-queue in-order completion (PE pc-monotone,
    ACT/DVE strict FIFO):  observing sem s >= v implies the v-th
    incrementing instruction and its whole same-queue prefix completed,
    hence all THEIR increments fired and all their waits were satisfied.
    """
    blocks = list(nc.m.functions[0].blocks)
    seq = []
    for blk in blocks:
        for ins in blk.instructions:
            seq.append(ins)

    def queue_key(ins):
        si = getattr(ins, "sync_info", None)
        nm = type(ins).__name__
        if nm in ("InstDMACopy", "InstTensorLoad", "InstTensorSave"):
            if si and si.on_update:
                return "Q" + si.on_update[0].ant_name
        return "E" + str(ins.engine)

    sem_count = {}
    incpoints = {}
    qpos = {}
    qidx = {}
    for ins in seq:
        qk = queue_key(ins)
        i = qpos.get(qk, 0)
        qidx[id(ins)] = (qk, i)
        qpos[qk] = i + 1
        si = getattr(ins, "sync_info", None)
        if si and si.on_update:
            for u in si.on_update:
                s = u.ant_name
                v = sem_count.get(s, 0) + (u.update_value or 1)
                sem_count[s] = v
                incpoints.setdefault(s, []).append((v, qk, i))

    per_queue = {}
    for ins in seq:
        qk, i = qidx[id(ins)]
        per_queue.setdefault(qk, []).append(ins)

    def merge(a, b):
        if not b:
            return a
        out = dict(a)
        for k, v in b.items():
            if out.get(k, 0) < v:
                out[k] = v
        return out

    comp_cache = {}

    def know_comp(qk, i):
        if i < 0:
            return {}
        key = (qk, i)
        if key in comp_cache:
            return comp_cache[key]
        know = dict(know_comp(qk, i - 1))
        ins = per_queue[qk][i]
        si = getattr(ins, "sync_info", None)
        if si:
            for w in (si.on_wait or []):
                if know.get(w.ant_name, 0) < w.wait_value:
                    know[w.ant_name] = w.wait_value
                    know = merge(know, know_from_obs(w.ant_name, w.wait_value))
        comp_cache[key] = know
        return know

    obs_cache = {}

    def _dma_sem(sem):
        return "DMA" in sem

    def know_from_obs(sem, v):
        if _dma_sem(sem):
            return {}
        key = (sem, v)
        if key in obs_cache:
            return obs_cache[key]
        obs_cache[key] = {}
        pts = incpoints.get(sem, [])
        know = {}
        if pts and all(q == pts[0][1] for _, q, _ in pts):
            for cnt, qk, i in pts:
                if cnt >= v:
                    if qk.startswith("E"):
                        know = dict(know_comp(qk, i))
                    know[sem] = cnt
                    break
        obs_cache[key] = know
        return know

    import os
    mode = os.environ.get("KERNEL_ELIM", "self")
    self_only = (mode == "self")

    def _same_queue_sem(sem, qk):
        pts = incpoints.get(sem, [])
        return bool(pts) and all(q == qk for _, q, _ in pts)

    dropped = 0
    kept = 0
    for qk, insts in per_queue.items():
        if not qk.startswith("E"):
            continue
        know = {}
        for ins in insts:
            si = getattr(ins, "sync_info", None)
            if not si:
                continue
            if type(ins).__name__ in ("InstDMACopy", "InstTensorLoad",
                                      "InstTensorSave", "InstTriggeredCopy"):
                continue
            waits = list(si.on_wait or [])
            if waits:
                changed = True
                waitset = waits[:]
                while changed:
                    changed = False
                    for w in waitset[:]:
                        if self_only and not _same_queue_sem(w.ant_name, qk):
                            continue
                        base = dict(know)
                        for w2 in waitset:
                            if w2 is w:
                                continue
                            base[w2.ant_name] = max(
                                base.get(w2.ant_name, 0), w2.wait_value)
                            base = merge(
                                base, know_from_obs(w2.ant_name, w2.wait_value))
                        if base.get(w.ant_name, 0) >= w.wait_value:
                            waitset.remove(w)
                            dropped += 1
                            changed = True
                            break
                for w in waitset:
                    kept += 1
                    know[w.ant_name] = max(know.get(w.ant_name, 0), w.wait_value)
                    know = merge(know, know_from_obs(w.ant_name, w.wait_value))
                if len(waitset) != len(waits):
                    ins.sync_info = mybir.SyncInfo(
                        on_wait=waitset, on_update=list(si.on_update or []))
    return dropped, kept


_SPLIT_WAIT_TYPES = {
    "InstMatmult", "InstTensorTensor", "InstTensorCopy", "InstActivation",
    "InstTensorScalarPtr", "InstTensorScalar", "InstReciprocal",
    "InstTensorReduce", "InstMemSet", "InstLdweights", "InstTranspose",
    "InstTensorTensorScan", "InstSelect", "InstCopy", "InstDMACopy",
    "InstTensorLoad", "InstTensorSave", "InstDrain",
}


def _split_matmul_waits(nc):
    """Walrus's TRN2 codegen allows at most one sync-wait per compute
    instruction.  Hoist every wait of a multi-wait instruction onto NoOps
    placed right before it on the same engine — the NX sequencer executes
    them in order, so semantics are identical.
    """
    cnt = 0
    for blk in nc.m.functions[0].blocks:
        insts = blk.instructions
        new = []
        for ins in insts:
            si = getattr(ins, "sync_info", None)
            if (type(ins).__name__ in _SPLIT_WAIT_TYPES and si is not None
                    and si.on_wait and len(si.on_wait) > 1):
                for j, w in enumerate(si.on_wait):
                    nop = mybir.InstNoOp(
                        name=f"{ins.name}-w{j}",
                        engine=ins.engine,
                        sync_info=mybir.SyncInfo(on_wait=[w], on_update=[]),
                        bass_nofuse=True,
                    )
                    new.append(nop)
                ins.sync_info = mybir.SyncInfo(
                    on_wait=[], on_update=list(si.on_update))
                cnt += 1
            new.append(ins)
        blk.instructions = new
    return cnt



def build_nc_full():
    nc = bass.Bass()
    # Per-core inputs.  x rows: pair p occupies partitions [0:128) as
    # (batch 2p on 0-63, batch 2p+1 on 64-127) after slicing [p*128:(p+1)*128).
    x32 = nc.declare_dram_parameter("x32", [BPC * C, N], F32, isOutput=False)
    x16 = nc.declare_dram_parameter("x16", [BPC * C, N], BF, isOutput=False)
    # four [64,64] conv weights, transposed, partition-duplicated, packed
    # along the free axis: [wth | wph | wgv | wma]
    wsmallT = nc.declare_dram_parameter("wsmallT", [128, 4 * C], BF,
                                        isOutput=False)
    # w_mk^T in k-quarter-major layout [4*N, 256]
    wmkhT = nc.declare_dram_parameter("wmkhT", [4 * N, 256], BF,
                                      isOutput=False)
    out_e = nc.declare_dram_parameter("out", [BPC * C, N], F32, isOutput=True)

    with tile.TileContext(nc) as tc:
        with (
            tc.tile_pool(name="consts", bufs=1) as consts,
            tc.tile_pool(name="acts", bufs=2) as acts,
            tc.tile_pool(name="bigacts", bufs=2) as bigacts,
            tc.tile_pool(name="psO", bufs=2, space="PSUM") as psO_pool,
            tc.tile_pool(name="psS", bufs=2, space="PSUM") as psS,
            tc.tile_pool(name="psSm", bufs=2, space="PSUM") as psSm,
        ):
            _build_body(nc, tc, consts, acts, bigacts, psO_pool, psS, psSm,
                        x32, x16, wsmallT, wmkhT, out_e)
    import os
    if os.environ.get("KERNEL_ELIM", "1") != "0":
        d, k = _eliminate_redundant_waits(nc)
        print(f"wait elimination: dropped {d}, kept {k}")
    _split_matmul_waits(nc)
    return nc


def _prep_weights(w_phi, w_theta, w_g, w_mask, w_mv, w_mk):
    bf = ml_dtypes.bfloat16

    def dup(a):  # [64, 64] -> [128, 64], duplicated on both partition halves
        return np.ascontiguousarray(np.concatenate([a, a], axis=0)).astype(bf)

    w_gv = (w_mv.astype(np.float64) @ w_g.astype(np.float64)).astype(np.float32)
    wsmall = np.concatenate(
        [dup(w_theta.T), dup(w_phi.T), dup(w_gv.T), dup(w_mask.T)], axis=1)
    # w_mk^T [m, k] -> k-quarter-major [4, m, 256] -> [4*m, 256]
    wmkT = np.ascontiguousarray(w_mk.T).astype(bf)
    wmkh = np.ascontiguousarray(
        wmkT.reshape(N, 4, 256).transpose(1, 0, 2)).reshape(4 * N, 256)
    return {
        "wsmallT": np.ascontiguousarray(wsmall),
        "wmkhT": wmkh,
    }


def kernel(x, w_phi, w_theta, w_g, w_mask, w_mv, w_mk, _trace=False):
    bf = ml_dtypes.bfloat16
    x = np.asarray(x, dtype=np.float32)
    weights = _prep_weights(np.asarray(w_phi, np.float32),
                            np.asarray(w_theta, np.float32),
                            np.asarray(w_g, np.float32),
                            np.asarray(w_mask, np.float32),
                            np.asarray(w_mv, np.float32),
                            np.asarray(w_mk, np.float32))

    xr = x.reshape(B, C, N)
    in_maps = []
    for i in range(NCORES):
        shard = np.ascontiguousarray(xr[i * BPC:(i + 1) * BPC]).reshape(BPC * C, N)
        m = {"x32": shard, "x16": shard.astype(bf)}
        m.update(weights)
        in_maps.append(m)

    nc = build_nc_full()
    res = run_bass_kernel_spmd(nc, in_maps, list(range(NCORES)), trace=_trace)
    outs = [np.asarray(res.results[i]["out"]).reshape(BPC, C, HH, WW)
            for i in range(NCORES)]
    full = np.concatenate(outs, axis=0)
    if _trace:
        return full, res
    return full



# revision 6
# speedup vs baseline: 1.1635x; 1.1635x over previous
"""Trainium2 Bass kernel for the non-local attention block (dense_transformer).

Reference computation per batch item b (x: [B=32, C=64, H=32, W=32], N=1024):
    xf    = x[b] reshaped [C, N]
    phi   = w_phi   @ xf                     [C, N]
    theta = (w_theta @ xf)^T                 [N, C]
    g     = (w_g @ xf)^T @ w_mv^T            [N, C]
    att   = theta @ phi                      [N, N]
    att   = att @ w_mk^T                     [N, N]
    att   = softmax(att, axis over rows n)
    out   = att @ g                          [N, C]
    final = w_mask @ out^T + xf              [C, N]

Key algebraic restructure: (theta @ phi) @ w_mk^T == theta @ (phi @ w_mk^T),
which removes the N^3 matmul (1073M MACs -> 2x67M MACs per batch).  The
softmax denominator divide is folded into the small g factor (64 wide).

Per-core layout (data-parallel, 4 batch items per core, processed as 2
stacked pairs occupying the 128 SBUF partitions; batch "b" on partitions
0-63, batch "c" on 64-127, PE quadrant tile-position packing runs both
batches' matmuls concurrently):
    T    = w_theta @ xf          [64, 1024]
    PhiT = xf^T @ w_phi^T        [1024, 64]
    GT   = xf^T @ (w_mv@w_g)^T   [1024, 64]
    P2   = PhiT^T @ w_mk^T       [64, 1024]  (accum over 8 m-chunks)
    S    = P2^T @ T              [1024, 1024] = att2^T
    E    = exp(S)  (ScalarE, fused row-sum via accum_out -> D)
    GTs  = GT * (1/D)            (fold softmax divide into g)
    O    = GTs^T @ E             [64, 1024]  (two n-half passes, accum over m)
    final= w_mask @ O + xf

Pipeline design (v2 — built from the v1 trace):
  * ScalarE exp is the hard floor (~45us busy per core); the kernel is
    organized so the ACT engine is continuously fed: psS pool has THREE
    [128,1024] slots so S production runs two exps ahead, and each chunk's
    8 S matmuls are emitted as one 4-quadrant-packed group.
  * The v1 head (21.6us to first exp) came from serial DMA issue plus all
    of stage-1/P2 being queued ahead of the first S chunk.  v2 issues x on
    the sync ring and wmk quarters on the GpSimd ring in parallel, warms
    the PE + preloads the exp ACT table immediately, and only PhiT/T/P2-q0
    gate the first exp (~6us).
  * PSUM (8 banks): psS 3x[128,1024] (6) | trans 2x[128,256] (1, P2
    quarters + mask pieces) | psO 1x[128,512] (1, O accumulates the two
    n-halves in sequential passes; h1 of pair p overlaps pair p+1).
  * Stage-1 (PhiT/T/GT) psums borrow psS slots as [128,1024] tiles (one
    cast each instead of many small-piece casts — DVE time matters).
  * The PE power/HAM duty cycle throttles sustained full-rate phases to
    K=4/8; per-chunk PE work (~3.5k cycles) is sized to roughly match the
    two-exp ACT window even at the throttled clock.

All matmul operands bf16; PSUM accumulation fp32; softmax sum in fp32 via
activation accum_out; residual add uses the bf16 x (adds ~0.3% rel err,
well within the 2e-2 budget).
"""

import numpy as np
import ml_dtypes

import concourse.bass as bass
import concourse.mybir as mybir
import concourse.tile as tile
from concourse.bass import _add_dep_helper
from concourse.bass_utils import run_bass_kernel_spmd

BF = mybir.dt.bfloat16
F32 = mybir.dt.float32
EXP = mybir.ActivationFunctionType.Exp

B, C, HH, WW = 32, 64, 32, 32
N = HH * WW          # 1024
NCORES = 8
BPC = B // NCORES    # 4 batch items per core
NPAIRS = BPC // 2    # 2 stacked pairs per core
NK = N // 128        # 8 chunks of 128 along the N dimension
NH = 512             # n-half


def _build_body(nc, tc, consts, acts, bigacts, psS, trans, psO,
                x16, wsmallT, wmkhT, out_e):
    lo = slice(0, 64)
    hi = slice(64, 128)

    def low():
        return tc.high_priority(offset=-100000)

    # ---- warmup: keep the PE busy from t=0 so the HAM K=8/8 flip lands
    # right as real work starts (~3.4us of activity needed).
    warm_in = consts.tile([128, 512], BF, tag="warm_in")
    nc.gpsimd.memset(warm_in[:], 0.0)
    warm_ps = psS.tile([128, N], F32, tag="psS", name="warm_ps")
    for i in range(16):
        nc.tensor.matmul(warm_ps[:, 0:256], lhsT=warm_in[:, 0:128],
                         rhs=warm_in[:, 0:256])

    # ---- input DMAs, split across the sync and GpSimd SWDGE rings so the
    # x transfer and the first wmk quarter land in parallel (~2.5us).
    wsmall = consts.tile([128, 4 * C], BF, tag="wsmall")
    nc.sync.dma_start(wsmall[:], wsmallT[:])
    wth = wsmall[:, 0 * C:1 * C]
    wph = wsmall[:, 1 * C:2 * C]
    wgv = wsmall[:, 2 * C:3 * C]
    wma = wsmall[:, 3 * C:4 * C]

    xball = acts.tile([128, NPAIRS, N], BF, tag="xball", bufs=1)
    nc.sync.dma_start(xball[:], x16.rearrange("(p q) n -> q p n", p=NPAIRS))

    # wmk^T in k-quarter-major DRAM layout [4, 1024(m), 256(k)]
    wmk_q = []
    for j in range(4):
        t = consts.tile([128, NK, 256], BF, tag=f"wmkq{j}")
        nc.gpsimd.dma_start(
            t[:], wmkhT[j * N:(j + 1) * N, :].rearrange(
                "(mc q) k -> q mc k", mc=NK))
        wmk_q.append(t)

    # ---- ACT table preload: a dummy exp moves the ~2.7us table load off
    # the first real exp (hidden under the DMA/stage-1 head).
    preload_e = acts.tile([128, 32], BF, tag="preload_e", bufs=1)
    preload_d = acts.tile([128, 1], F32, tag="preload_d", bufs=1)
    nc.scalar.activation(preload_e[:], warm_in[:, 0:32], EXP,
                         accum_out=preload_d[:])

    st = [dict() for _ in range(NPAIRS)]

    def stage1_phit(p):
        """PhiT for pair p: [128(m-part), 64(c)] chunks; b in cols 0:512,
        c in cols 512:1024 of one [128,1024] psS-slot tile."""
        xb = xball[:, p, :]
        s = st[p]
        ps = psS.tile([128, N], F32, tag="psS", name="psPhiT")
        for m in range(NK):
            mm = slice(m * 128, (m + 1) * 128)
            nc.tensor.matmul(ps[:, m * C:(m + 1) * C],
                             lhsT=xb[lo, mm], rhs=wph[lo, :])
            nc.tensor.matmul(ps[:, NH + m * C:NH + (m + 1) * C],
                             lhsT=xb[hi, mm], rhs=wph[hi, :])
        PhiT = acts.tile([128, N], BF, tag="PhiT", name="PhiT")
        nc.vector.tensor_copy(out=PhiT[:], in_=ps[:])
        s["PhiT"] = PhiT

    def stage1_t(p):
        xb = xball[:, p, :]
        s = st[p]
        ps = psS.tile([128, N], F32, tag="psS", name="psT")
        for h in range(2):
            hh = slice(h * NH, (h + 1) * NH)
            nc.tensor.matmul(ps[lo, hh], lhsT=wth[lo, :], rhs=xb[lo, hh])
            nc.tensor.matmul(ps[hi, hh], lhsT=wth[hi, :], rhs=xb[hi, hh])
        T_sb = acts.tile([128, N], BF, tag="T_sb", name="T_sb")
        nc.vector.tensor_copy(out=T_sb[:], in_=ps[:])
        s["T_sb"] = T_sb

    def stage1_gt(p):
        xb = xball[:, p, :]
        s = st[p]
        ps = psS.tile([128, N], F32, tag="psS", name="psGT")
        for m in range(NK):
            mm = slice(m * 128, (m + 1) * 128)
            nc.tensor.matmul(ps[:, m * C:(m + 1) * C],
                             lhsT=xb[lo, mm], rhs=wgv[lo, :])
            nc.tensor.matmul(ps[:, NH + m * C:NH + (m + 1) * C],
                             lhsT=xb[hi, mm], rhs=wgv[hi, :])
        GT = acts.tile([128, N], BF, tag="GT", name="GT")
        nc.vector.tensor_copy(out=GT[:], in_=ps[:])
        s["GT"] = GT
        s["GTs"] = acts.tile([128, N], BF, tag="GTs", name="GTs")

    def alloc_pair(p):
        s = st[p]
        s["P2"] = acts.tile([128, N], BF, tag="P2", name="P2")
        s["E_b"] = bigacts.tile([128, NK, N], BF, tag="E_b", name="E_b")
        s["E_c"] = bigacts.tile([128, NK, N], BF, tag="E_c", name="E_c")
        s["D"] = acts.tile([128, 2 * NK], F32, tag="D", name="D")
        s["R"] = acts.tile([128, 2 * NK], F32, tag="R", name="R")
        s["O_sb"] = acts.tile([128, N], BF, tag="O_sb", name="O_sb")

    def p2_quarter(p, j):
        """P2 column-quarter j (256 k's) for pair p, col-split by batch."""
        s = st[p]
        ps = trans.tile([128, 256], F32, tag="tr", name="psP2")
        for m in range(NK):
            cc = slice(m * C, (m + 1) * C)
            ncc = slice(NH + m * C, NH + (m + 1) * C)
            nc.tensor.matmul(ps[lo, :], lhsT=s["PhiT"][:, cc],
                             rhs=wmk_q[j][:, m, :],
                             start=(m == 0), stop=(m == NK - 1))
            nc.tensor.matmul(ps[hi, :], lhsT=s["PhiT"][:, ncc],
                             rhs=wmk_q[j][:, m, :],
                             start=(m == 0), stop=(m == NK - 1))
        nc.vector.tensor_copy(out=s["P2"][:, j * 256:(j + 1) * 256], in_=ps[:])

    def s_group(p, k):
        """All 8 S matmuls for chunk k (both batches), 4-quadrant packed."""
        s = st[p]
        klo = slice(k * 128, k * 128 + 64)
        khi = slice(k * 128 + 64, (k + 1) * 128)
        sb = psS.tile([128, N], F32, tag="psS", name="psS_b")
        sc = psS.tile([128, N], F32, tag="psS", name="psS_c")
        last = None
        for h in range(2):
            hh = slice(h * NH, (h + 1) * NH)
            nc.tensor.matmul(sb[lo, hh], lhsT=s["P2"][lo, klo],
                             rhs=s["T_sb"][lo, hh])
            nc.tensor.matmul(sb[hi, hh], lhsT=s["P2"][lo, khi],
                             rhs=s["T_sb"][lo, hh])
            nc.tensor.matmul(sc[lo, hh], lhsT=s["P2"][hi, klo],
                             rhs=s["T_sb"][hi, hh])
            last = nc.tensor.matmul(sc[hi, hh], lhsT=s["P2"][hi, khi],
                                    rhs=s["T_sb"][hi, hh])
        return sb, sc, last

    def exp_chunk(p, k, sb, sc):
        s = st[p]
        nc.scalar.activation(s["E_b"][:, k, :], sb[:], EXP,
                             accum_out=s["D"][:, 2 * k:2 * k + 1])
        nc.scalar.activation(s["E_c"][:, k, :], sc[:], EXP,
                             accum_out=s["D"][:, 2 * k + 1:2 * k + 2])

    def gts_chunk(p, k):
        s = st[p]
        cc = slice(k * C, (k + 1) * C)
        ncc = slice(NH + k * C, NH + (k + 1) * C)
        nc.vector.reciprocal(s["R"][:, 2 * k:2 * k + 2],
                             s["D"][:, 2 * k:2 * k + 2])
        nc.vector.tensor_scalar_mul(s["GTs"][:, cc], s["GT"][:, cc],
                                    s["R"][:, 2 * k:2 * k + 1])
        nc.vector.tensor_scalar_mul(s["GTs"][:, ncc], s["GT"][:, ncc],
                                    s["R"][:, 2 * k + 1:2 * k + 2])

    def o_pass_init(p, h):
        st[p][f"psO{h}"] = psO.tile([128, NH], F32, tag="psO",
                                    name=f"psO_p{p}h{h}")

    def o_chunk(p, m, h, after=None):
        """O accumulation chunk m for n-half h (both batches, col-split)."""
        s = st[p]
        cc = slice(m * C, (m + 1) * C)
        ncc = slice(NH + m * C, NH + (m + 1) * C)
        hh = slice(h * NH, (h + 1) * NH)
        ps = s[f"psO{h}"]
        mm1 = nc.tensor.matmul(ps[lo, :], lhsT=s["GTs"][:, cc],
                               rhs=s["E_b"][:, m, hh],
                               start=(m == 0), stop=(m == NK - 1))
        if after is not None:
            _add_dep_helper(mm1.ins, after.ins, reason="O after next S group")
        nc.tensor.matmul(ps[hi, :], lhsT=s["GTs"][:, ncc],
                         rhs=s["E_c"][:, m, hh],
                         start=(m == 0), stop=(m == NK - 1))

    def o_copyback(p, h):
        s = st[p]
        hh = slice(h * NH, (h + 1) * NH)
        nc.vector.tensor_copy(out=s["O_sb"][:, hh], in_=s[f"psO{h}"][:])

    def finish(p):
        """mask conv pieces + residual add + out DMA for pair p."""
        s = st[p]
        out_sb = acts.tile([128, N], F32, tag="out_sb", name="out_sb")
        for t in range(2):
            nn = slice(t * NH, (t + 1) * NH)
            psM = trans.tile([128, NH], F32, tag="tr", name="psM")
            nc.tensor.matmul(psM[lo, :], lhsT=wma[lo, :], rhs=s["O_sb"][lo, nn])
            nc.tensor.matmul(psM[hi, :], lhsT=wma[hi, :], rhs=s["O_sb"][hi, nn])
            nc.vector.tensor_tensor(out_sb[:, nn], psM[:],
                                    xball[:, p, nn], mybir.AluOpType.add)
        nc.gpsimd.dma_start(out_e[p * 128:(p + 1) * 128, :], out_sb[:])

    # ================= emission schedule =================
    stage1_phit(0)
    alloc_pair(0)
    p2_quarter(0, 0)
    stage1_t(0)
    with low():
        stage1_gt(0)

    for p in range(NPAIRS):
        nxt = p + 1
        o_pass_init(p, 0)
        for k in range(NK):
            sb, sc, s_last = s_group(p, k)
            exp_chunk(p, k, sb, sc)
            gts_chunk(p, k)
            if k >= 1:
                o_chunk(p, k - 1, 0, after=s_last)
            # ---- fillers, low priority ----
            with low():
                if p == 0:
                    if k == 0:
                        p2_quarter(0, 1)
                    if k == 1:
                        p2_quarter(0, 2)
                    if k == 2:
                        p2_quarter(0, 3)
                if nxt < NPAIRS:
                    if k == 3:
                        stage1_phit(nxt)
                        alloc_pair(nxt)
                    if k == 4:
                        stage1_t(nxt)
                        stage1_gt(nxt)
                    if 5 <= k <= 7:
                        p2_quarter(nxt, k - 5)
                if p > 0:
                    # previous pair: finish h0, run its h1 pass, mask, out
                    if k == 0:
                        o_chunk(p - 1, NK - 1, 0)
                        o_copyback(p - 1, 0)
                        o_pass_init(p - 1, 1)
                        for m in range(0, 3):
                            o_chunk(p - 1, m, 1)
                    if k == 1:
                        for m in range(3, 6):
                            o_chunk(p - 1, m, 1)
                    if k == 2:
                        for m in range(6, NK):
                            o_chunk(p - 1, m, 1)
                        o_copyback(p - 1, 1)
                    if k == 3:
                        finish(p - 1)
        if nxt < NPAIRS:
            with low():
                p2_quarter(nxt, 3)

    # ---- tail: last pair's h0 leftover + h1 pass + finish
    p = NPAIRS - 1
    o_chunk(p, NK - 1, 0)
    o_copyback(p, 0)
    o_pass_init(p, 1)
    for m in range(NK):
        o_chunk(p, m, 1)
    o_copyback(p, 1)
    finish(p)


def _eliminate_redundant_waits(nc):
    """Transitive redundant-wait elimination over the final BIR stream.

    Tile's sem assignment is per-proc minimal but NOT transitively minimal:
    e.g. a matmul reusing a PSUM slot gets both (ACT >= k) [reader done] and
    (PE >= p) [previous writer done] waits, although observing ACT >= k
    already implies PE >= p (the reader waited on the writer).  The extra
    same-engine waits serialize the PE pipeline (no back-to-back streaming,
    no quadrant concurrency).

    Soundness relies on per-queue in-order completion (PE pc-monotone,
    ACT/DVE strict FIFO):  observing sem s >= v implies the v-th
    incrementing instruction and its whole same-queue prefix completed,
    hence all THEIR increments fired and all their waits were satisfied.
    """
    blocks = list(nc.m.functions[0].blocks)
    seq = []
    for blk in blocks:
        for ins in blk.instructions:
            seq.append(ins)

    def queue_key(ins):
        si = getattr(ins, "sync_info", None)
        nm = type(ins).__name__
        if nm in ("InstDMACopy", "InstTensorLoad", "InstTensorSave"):
            if si and si.on_update:
                return "Q" + si.on_update[0].ant_name
        return "E" + str(ins.engine)

    sem_count = {}
    incpoints = {}
    qpos = {}
    qidx = {}
    for ins in seq:
        qk = queue_key(ins)
        i = qpos.get(qk, 0)
        qidx[id(ins)] = (qk, i)
        qpos[qk] = i + 1
        si = getattr(ins, "sync_info", None)
        if si and si.on_update:
            for u in si.on_update:
                s = u.ant_name
                v = sem_count.get(s, 0) + (u.update_value or 1)
                sem_count[s] = v
                incpoints.setdefault(s, []).append((v, qk, i))

    per_queue = {}
    for ins in seq:
        qk, i = qidx[id(ins)]
        per_queue.setdefault(qk, []).append(ins)

    def merge(a, b):
        if not b:
            return a
        out = dict(a)
        for k, v in b.items():
            if out.get(k, 0) < v:
                out[k] = v
        return out

    comp_cache = {}

    def know_comp(qk, i):
        if i < 0:
            return {}
        key = (qk, i)
        if key in comp_cache:
            return comp_cache[key]
        know = dict(know_comp(qk, i - 1))
        ins = per_queue[qk][i]
        si = getattr(ins, "sync_info", None)
        if si:
            for w in (si.on_wait or []):
                if know.get(w.ant_name, 0) < w.wait_value:
                    know[w.ant_name] = w.wait_value
                    know = merge(know, know_from_obs(w.ant_name, w.wait_value))
        comp_cache[key] = know
        return know

    obs_cache = {}

    def _dma_sem(sem):
        return "DMA" in sem

    def know_from_obs(sem, v):
        if _dma_sem(sem):
            return {}
        key = (sem, v)
        if key in obs_cache:
            return obs_cache[key]
        obs_cache[key] = {}
        pts = incpoints.get(sem, [])
        know = {}
        if pts and all(q == pts[0][1] for _, q, _ in pts):
            for cnt, qk, i in pts:
                if cnt >= v:
                    if qk.startswith("E"):
                        know = dict(know_comp(qk, i))
                    know[sem] = cnt
                    break
        obs_cache[key] = know
        return know

    import os
    mode = os.environ.get("KERNEL_ELIM", "self")
    self_only = (mode == "self")

    def _same_queue_sem(sem, qk):
        pts = incpoints.get(sem, [])
        return bool(pts) and all(q == qk for _, q, _ in pts)

    dropped = 0
    kept = 0
    for qk, insts in per_queue.items():
        if not qk.startswith("E"):
            continue
        know = {}
        for ins in insts:
            si = getattr(ins, "sync_info", None)
            if not si:
                continue
            if type(ins).__name__ in ("InstDMACopy", "InstTensorLoad",
                                      "InstTensorSave", "InstTriggeredCopy"):
                continue
            waits = list(si.on_wait or [])
            if waits:
                changed = True
                waitset = waits[:]
                while changed:
                    changed = False
                    for w in waitset[:]:
                        if self_only and not _same_queue_sem(w.ant_name, qk):
                            continue
                        base = dict(know)
                        for w2 in waitset:
                            if w2 is w:
                                continue
                            base[w2.ant_name] = max(
                                base.get(w2.ant_name, 0), w2.wait_value)
                            base = merge(
                                base, know_from_obs(w2.ant_name, w2.wait_value))
                        if base.get(w.ant_name, 0) >= w.wait_value:
                            waitset.remove(w)
                            dropped += 1
                            changed = True
                            break
                for w in waitset:
                    kept += 1
                    know[w.ant_name] = max(know.get(w.ant_name, 0), w.wait_value)
                    know = merge(know, know_from_obs(w.ant_name, w.wait_value))
                if len(waitset) != len(waits):
                    ins.sync_info = mybir.SyncInfo(
                        on_wait=waitset, on_update=list(si.on_update or []))
    return dropped, kept


_SPLIT_WAIT_TYPES = {
    "InstMatmult", "InstTensorTensor", "InstTensorCopy", "InstActivation",
    "InstTensorScalarPtr", "InstTensorScalar", "InstReciprocal",
    "InstTensorReduce", "InstMemSet", "InstLdweights", "InstTranspose",
    "InstTensorTensorScan", "InstSelect", "InstCopy", "InstDMACopy",
    "InstTensorLoad", "InstTensorSave", "InstDrain",
}


def _split_matmul_waits(nc):
    """Walrus's TRN2 codegen allows at most one sync-wait per compute
    instruction.  Hoist every wait of a multi-wait instruction onto NoOps
    placed right before it on the same engine — the NX sequencer executes
    them in order, so semantics are identical.
    """
    cnt = 0
    for blk in nc.m.functions[0].blocks:
        insts = blk.instructions
        new = []
        for ins in insts:
            si = getattr(ins, "sync_info", None)
            if (type(ins).__name__ in _SPLIT_WAIT_TYPES and si is not None
                    and si.on_wait and len(si.on_wait) > 1):
                for j, w in enumerate(si.on_wait):
                    nop = mybir.InstNoOp(
                        name=f"{ins.name}-w{j}",
                        engine=ins.engine,
                        sync_info=mybir.SyncInfo(on_wait=[w], on_update=[]),
                        bass_nofuse=True,
                    )
                    new.append(nop)
                ins.sync_info = mybir.SyncInfo(
                    on_wait=[], on_update=list(si.on_update))
                cnt += 1
            new.append(ins)
        blk.instructions = new
    return cnt


def build_nc_full():
    nc = bass.Bass()
    # Per-core inputs.  x rows: pair p occupies partitions [0:128) as
    # (batch 2p on 0-63, batch 2p+1 on 64-127) after slicing.
    x16 = nc.declare_dram_parameter("x16", [BPC * C, N], BF, isOutput=False)
    wsmallT = nc.declare_dram_parameter("wsmallT", [128, 4 * C], BF,
                                        isOutput=False)
    wmkhT = nc.declare_dram_parameter("wmkhT", [4 * N, 256], BF,
                                      isOutput=False)
    out_e = nc.declare_dram_parameter("out", [BPC * C, N], F32, isOutput=True)

    with tile.TileContext(nc) as tc:
        with (
            tc.tile_pool(name="consts", bufs=1) as consts,
            tc.tile_pool(name="acts", bufs=2) as acts,
            tc.tile_pool(name="bigacts", bufs=2) as bigacts,
            tc.tile_pool(name="psS", bufs=3, space="PSUM") as psS,
            tc.tile_pool(name="trans", bufs=1, space="PSUM") as trans,
            tc.tile_pool(name="psO", bufs=1, space="PSUM") as psO,
        ):
            _build_body(nc, tc, consts, acts, bigacts, psS, trans, psO,
                        x16, wsmallT, wmkhT, out_e)
    import os
    if os.environ.get("KERNEL_ELIM", "1") != "0":
        d, k = _eliminate_redundant_waits(nc)
        print(f"wait elimination: dropped {d}, kept {k}")
    _split_matmul_waits(nc)
    return nc


def _prep_weights(w_phi, w_theta, w_g, w_mask, w_mv, w_mk):
    bf = ml_dtypes.bfloat16

    def dup(a):  # [64, 64] -> [128, 64], duplicated on both partition halves
        return np.ascontiguousarray(np.concatenate([a, a], axis=0)).astype(bf)

    w_gv = (w_mv.astype(np.float64) @ w_g.astype(np.float64)).astype(np.float32)
    wsmall = np.concatenate(
        [dup(w_theta.T), dup(w_phi.T), dup(w_gv.T), dup(w_mask.T)], axis=1)
    # w_mk^T [m, k] -> k-quarter-major [4, m, 256] -> [4*m, 256]
    wmkT = np.ascontiguousarray(w_mk.T).astype(bf)
    wmkh = np.ascontiguousarray(
        wmkT.reshape(N, 4, 256).transpose(1, 0, 2)).reshape(4 * N, 256)
    return {
        "wsmallT": np.ascontiguousarray(wsmall),
        "wmkhT": wmkh,
    }


def kernel(x, w_phi, w_theta, w_g, w_mask, w_mv, w_mk, _trace=False):
    bf = ml_dtypes.bfloat16
    x = np.asarray(x, dtype=np.float32)
    weights = _prep_weights(np.asarray(w_phi, np.float32),
                            np.asarray(w_theta, np.float32),
                            np.asarray(w_g, np.float32),
                            np.asarray(w_mask, np.float32),
                            np.asarray(w_mv, np.float32),
                            np.asarray(w_mk, np.float32))

    xr = x.reshape(B, C, N)
    in_maps = []
    for i in range(NCORES):
        shard = np.ascontiguousarray(xr[i * BPC:(i + 1) * BPC]).reshape(BPC * C, N)
        m = {"x16": shard.astype(bf)}
        m.update(weights)
        in_maps.append(m)

    nc = build_nc_full()
    res = run_bass_kernel_spmd(nc, in_maps, list(range(NCORES)), trace=_trace)
    outs = [np.asarray(res.results[i]["out"]).reshape(BPC, C, HH, WW)
            for i in range(NCORES)]
    full = np.concatenate(outs, axis=0)
    if _trace:
        return full, res
    return full
